# revision 15
# baseline (speedup 1.0000x reference)
"""Trainium2 Bass kernel for EnhancedGNNJobRecommender (3x TransformerConv + BN + MLP heads).

Self-contained: host preprocessing (edge sort/shard), Bass/Tile kernel builder,
SPMD runner over 8 NeuronCores, output assembly.

Sharding: edges sorted by dst, dst-range-aligned across cores (SL nodes/core);
segment softmax + scatter are core-local. Between layers the pre-BN activation
slices are AllGathered; BN is folded into the next layer's weights on device
(stats via a tiny AllReduce), so BN is never materialized.

Per layer on each core:
  node phase A (rolled): kv_table[n] = h_full[n] @ [Wk'|Wv']     (full table)
  node phase B (rolled): q_sl/s_sl  = h_own @ Wq' + bq' / Ws' + bs'
  edge phase (unrolled blocks of BSZ chunks x 128 edges):
    kv = gather(kv_table, src); ekv = eaT_aug^T @ [We;bk' | We;bv']
    kj|vj = kv + ekv; qe = gather(q_sl, dstl)
    alpha = rowsum(qe*kj per head)/8; ex = exp(alpha)  (no max subtraction)
    rhs = [ex*vj | ex]; OH[e,n] = (dsh == iota)
    acc_psum[tile] += OH^T @ rhs    (softmax numer+denom in one matmul)
  tile epilogue: att = numer/(denom+eps); h_out = leaky(att + s); stats +=
"""
import sys
import numpy as np

sys.path.insert(0, "/opt/trn_rl_repo")

# ---------------- problem constants ----------------
N_JOBS = 2000
N_NODES = 50000
IN_DIM = 128
EDGE_DIM = 3
HEADS = 4
HEAD_DIM = 64
HID = 256
BN_EPS = 1e-5
SM_EPS = 1e-16
NEG = 0.2
NC = 8
P = 128
BSZ = 6          # chunks per edge-phase block (6*128 = 768 edges)
INV_SQRT_D = float(1.0 / np.sqrt(np.float32(HEAD_DIM)))


# ---------------- host preprocessing ----------------
def _prep_edges(edge_index, edge_attr, n_nodes, n_cores):
    """Sort by dst, shard by dst range, pad per tile to 128-multiples with a
    chunk->tile map uniform across cores."""
    SL = n_nodes // n_cores
    TPC = (SL + P - 1) // P
    src = np.asarray(edge_index)[0].astype(np.int64)
    dst = np.asarray(edge_index)[1].astype(np.int64)
    ea = np.asarray(edge_attr).astype(np.float32)
    perm = np.argsort(dst, kind="stable")
    s_src, s_dst, s_ea = src[perm], dst[perm], ea[perm]

    bounds = np.searchsorted(s_dst, np.arange(0, n_nodes + 1, P))
    if len(bounds) < n_cores * TPC + 1:   # n_nodes % P != 0 handled by TPC*P
        bounds = np.searchsorted(s_dst, np.concatenate(
            [np.arange(0, n_cores * SL, P), [n_nodes]]))
    # tile bounds aligned per core (tiles never straddle cores since SL%P may !=0)
    tile_lo = np.concatenate([np.arange(c * SL, (c + 1) * SL, P)[:TPC]
                              for c in range(n_cores)])
    tile_hi = np.concatenate([np.minimum(np.arange(c * SL, (c + 1) * SL, P)[:TPC] + P,
                                         (c + 1) * SL) for c in range(n_cores)])
    e_lo = np.searchsorted(s_dst, tile_lo).reshape(n_cores, TPC)
    e_hi = np.searchsorted(s_dst, tile_hi).reshape(n_cores, TPC)
    cnt = e_hi - e_lo
    m_t = np.maximum(((cnt + P - 1) // P).max(axis=0), 1)       # chunks per tile
    nch = int(m_t.sum())
    chunk_tile = np.repeat(np.arange(TPC), m_t)

    per_core = []
    for c in range(n_cores):
        lo = c * SL
        csrc = np.zeros((nch, P), np.int32)
        cdstl = np.zeros((nch, P), np.int32)
        cdsh = np.full((nch, P), -1000.0, np.float32)
        cea = np.zeros((nch, EDGE_DIM + 1, P), np.float32)
        ci = 0
        for t in range(TPC):
            e0, e1 = e_lo[c, t], e_hi[c, t]
            n = e1 - e0
            ts_, td_, te_ = s_src[e0:e1], s_dst[e0:e1], s_ea[e0:e1]
            for j in range(m_t[t]):
                a, b = j * P, min((j + 1) * P, n)
                k = max(0, b - a)
                if k > 0:
                    csrc[ci, :k] = ts_[a:b]
                    cdstl[ci, :k] = td_[a:b] - lo
                    cdsh[ci, :k] = (td_[a:b] - (lo + t * P)).astype(np.float32)
                    cea[ci, :EDGE_DIM, :k] = te_[a:b].T
                    cea[ci, EDGE_DIM, :k] = 1.0
                ci += 1
        assert ci == nch
        per_core.append({
            "src_pm": np.ascontiguousarray(csrc.T),             # [128, nch] i32
            "dstl_pm": np.ascontiguousarray(cdstl.T),           # [128, nch] i32
            "dsh_pm": np.ascontiguousarray(cdsh.T),             # [128, nch] f32
            "ea_pm": np.ascontiguousarray(
                cea.transpose(1, 0, 2).reshape(EDGE_DIM + 1, nch * P)),
        })
    meta = {"SL": SL, "TPC": TPC, "nch": nch, "chunk_tile": chunk_tile,
            "m_t": m_t, "n_cores": n_cores, "n_nodes": n_nodes}
    return meta, per_core


def _np_params(params):
    def cv(v):
        return np.ascontiguousarray(np.asarray(v), dtype=np.float32)
    out = {}
    for k, v in params.items():
        if isinstance(v, dict):
            out[k] = {kk: cv(vv) for kk, vv in v.items()}
        else:
            out[k] = [{kk: cv(vv) for kk, vv in lp.items()} for lp in v]
    return out


# ---------------- kernel builder ----------------
def build_kernel(meta, n_layers=3, with_heads=True):
    import concourse.bass as bass
    import concourse.mybir as mybir
    import concourse.tile as tile
    from concourse import bacc
    from concourse.bass import IndirectOffsetOnAxis

    F32 = mybir.dt.float32
    I32 = mybir.dt.int32
    AF = mybir.ActivationFunctionType
    OP = mybir.AluOpType

    SL, TPC, nch = meta["SL"], meta["TPC"], meta["nch"]
    chunk_tile, m_t = meta["chunk_tile"], meta["m_t"]
    n_cores, n_nodes = meta["n_cores"], meta["n_nodes"]
    NT_full = n_nodes // P             # full kv tiles
    rem_rows = n_nodes - NT_full * P   # partial kv tile rows (80 for 50000)
    SLP = TPC * P
    groups = [list(range(n_cores))]
    last_rows = SL - (TPC - 1) * P
    NBLK = (nch + BSZ - 1) // BSZ

    nc = bacc.Bacc("TRN2", target_bir_lowering=False, debug=False)

    # ---- inputs ----
    x_full = nc.dram_tensor("x_full", [n_nodes, IN_DIM], F32, kind="ExternalInput")
    x_own = nc.dram_tensor("x_own", [SLP, IN_DIM], F32, kind="ExternalInput")
    src_pm = nc.dram_tensor("src_pm", [P, nch], I32, kind="ExternalInput")
    dstl_pm = nc.dram_tensor("dstl_pm", [P, nch], I32, kind="ExternalInput")
    dsh_pm = nc.dram_tensor("dsh_pm", [P, nch], F32, kind="ExternalInput")
    ea_pm = nc.dram_tensor("ea_pm", [EDGE_DIM + 1, nch * P], F32, kind="ExternalInput")
    iota_in = nc.dram_tensor("iota_in", [P, P], F32, kind="ExternalInput")
    ident_in = nc.dram_tensor("ident_in", [P, P], F32, kind="ExternalInput")
    maskcol_in = nc.dram_tensor("maskcol_in", [P, 2], F32, kind="ExternalInput")

    win = {}
    for li in range(n_layers):
        fi = IN_DIM if li == 0 else HID
        for wn in ("Wq", "Wk", "Wv", "Ws"):
            win[f"{wn}{li}"] = nc.dram_tensor(f"{wn}{li}", [fi, HID], F32, kind="ExternalInput")
        for bn_ in ("bq", "bs"):
            win[f"{bn_}{li}"] = nc.dram_tensor(f"{bn_}{li}", [1, HID], F32, kind="ExternalInput")
        win[f"Wek{li}"] = nc.dram_tensor(f"Wek{li}", [EDGE_DIM + 1, HID], F32, kind="ExternalInput")
        win[f"Wev{li}"] = nc.dram_tensor(f"Wev{li}", [EDGE_DIM + 1, HID], F32, kind="ExternalInput")
        if li > 0:
            win[f"g{li - 1}"] = nc.dram_tensor(f"g{li - 1}", [1, HID], F32, kind="ExternalInput")
            win[f"beta{li - 1}"] = nc.dram_tensor(f"beta{li - 1}", [1, HID], F32, kind="ExternalInput")
    lln = n_layers - 1
    win[f"g{lln}"] = nc.dram_tensor(f"g{lln}", [1, HID], F32, kind="ExternalInput")
    win[f"beta{lln}"] = nc.dram_tensor(f"beta{lln}", [1, HID], F32, kind="ExternalInput")
    if with_heads:
        hd = {"job": [(HID, 2 * HEAD_DIM), (2 * HEAD_DIM, HEAD_DIM), (HEAD_DIM, N_JOBS)],
              "demand": [(HID, HEAD_DIM), (HEAD_DIM, 1)],
              "hot": [(HID, HEAD_DIM), (HEAD_DIM, 1)]}
        for hn, dims in hd.items():
            for i, (a, b) in enumerate(dims):
                win[f"{hn}W{i}"] = nc.dram_tensor(f"{hn}W{i}", [a, b], F32, kind="ExternalInput")
                win[f"{hn}b{i}"] = nc.dram_tensor(f"{hn}b{i}", [1, b], F32, kind="ExternalInput")

    # ---- outputs ----
    h_out_ext = nc.dram_tensor("h_last", [SL, HID], F32, kind="ExternalOutput")
    if with_heads:
        jl_out = nc.dram_tensor("job_logits", [N_JOBS, N_JOBS], F32, kind="ExternalOutput")
        dm_out = nc.dram_tensor("demand", [N_JOBS, 1], F32, kind="ExternalOutput")
        ht_out = nc.dram_tensor("hot", [N_JOBS, 1], F32, kind="ExternalOutput")

    # ---- internal DRAM ----
    kv_table = nc.dram_tensor("kv_table", [n_nodes, 2 * HID], F32)
    q_sl = nc.dram_tensor("q_sl", [SLP, HID], F32)
    s_sl = nc.dram_tensor("s_sl", [SLP, HID], F32)
    ag_in = [nc.dram_tensor(f"ag_in{li}", [SL, HID], F32) for li in range(n_layers)]
    ag_out = [nc.dram_tensor(f"ag_out{li}", [n_nodes, HID], F32, addr_space="Shared")
              for li in range(max(0, n_layers - 1))]
    ar_in = [nc.dram_tensor(f"ar_in{li}", [1, 2 * HID], F32) for li in range(n_layers)]
    ar_out = [nc.dram_tensor(f"ar_out{li}", [1, 2 * HID], F32, addr_space="Shared")
              for li in range(n_layers)]

    with tile.TileContext(nc) as tc:
        from contextlib import ExitStack
        ctx = ExitStack()
        cst = ctx.enter_context(tc.tile_pool(name="cst", bufs=1))
        wpool = ctx.enter_context(tc.tile_pool(name="wpool", bufs=1))
        nodeb = ctx.enter_context(tc.tile_pool(name="nodeb", bufs=2))
        nps = ctx.enter_context(tc.tile_pool(name="nps", bufs=2, space="PSUM"))
        tpps = ctx.enter_context(tc.tile_pool(name="smps", bufs=1, space="PSUM"))
        edgeb = ctx.enter_context(tc.tile_pool(name="edgeb", bufs=2))
        ekvps = nps
        accps = nps
        stps = tpps
        fold = ctx.enter_context(tc.tile_pool(name="fold", bufs=1))

        ident = cst.tile([P, P], F32)
        nc.sync.dma_start(out=ident[:], in_=ident_in[:])
        iota_t = cst.tile([P, P], F32)
        nc.sync.dma_start(out=iota_t[:], in_=iota_in[:])
        maskcol = cst.tile([P, 2], F32)
        nc.sync.dma_start(out=maskcol[:], in_=maskcol_in[:])
        ones_row = cst.tile([1, P], F32)
        nc.any.memset(ones_row[:], 1.0)
        one_t = cst.tile([1, 1], F32)
        nc.any.memset(one_t[:], 1.0)
        eps_t = cst.tile([1, 1], F32)
        nc.any.memset(eps_t[:], BN_EPS)

        wq_f = wpool.tile([P, 2, HID], F32, tag="wq_f")
        wk_f = wpool.tile([P, 2, HID], F32, tag="wk_f")
        wv_f = wpool.tile([P, 2, HID], F32, tag="wv_f")
        ws_f = wpool.tile([P, 2, HID], F32, tag="ws_f")
        wek_t = wpool.tile([EDGE_DIM + 1, HID], F32, tag="wek")
        wev_t = wpool.tile([EDGE_DIM + 1, HID], F32, tag="wev")
        bq_t = wpool.tile([1, HID], F32, tag="bq")
        bs_t = wpool.tile([1, HID], F32, tag="bs")

        def stage_weights_raw(li, fi):
            kc = fi // P
            for wt, name in ((wq_f, "Wq"), (wk_f, "Wk"), (wv_f, "Wv"), (ws_f, "Ws")):
                nc.sync.dma_start(
                    out=wt[:, 0:kc, :],
                    in_=win[f"{name}{li}"][:].rearrange("(c p) g -> p c g", p=P))
            nc.sync.dma_start(out=wek_t[:], in_=win[f"Wek{li}"][:])
            nc.sync.dma_start(out=wev_t[:], in_=win[f"Wev{li}"][:])
            nc.sync.dma_start(out=bq_t[:], in_=win[f"bq{li}"][:])
            nc.sync.dma_start(out=bs_t[:], in_=win[f"bs{li}"][:])

        def bn_scalars(stats_src, gname, bname, tagp):
            """Compute BN fold rows a,b [1,HID] + columns [(128,1)x2] from stats."""
            st = fold.tile([1, 2 * HID], F32, tag="fst")
            nc.sync.dma_start(out=st[:], in_=stats_src[:])
            m_row = fold.tile([1, HID], F32, tag="fm")
            nc.scalar.activation(out=m_row[:], in_=st[0:1, 0:HID], func=AF.Copy,
                                 scale=1.0 / n_nodes)
            v_row = fold.tile([1, HID], F32, tag="fv")
            nc.scalar.activation(out=v_row[:], in_=st[0:1, HID:2 * HID], func=AF.Copy,
                                 scale=1.0 / n_nodes)
            msq = fold.tile([1, HID], F32, tag="fmsq")
            nc.vector.tensor_tensor(out=msq[:], in0=m_row[:], in1=m_row[:], op=OP.mult)
            nc.vector.tensor_tensor(out=v_row[:], in0=v_row[:], in1=msq[:], op=OP.subtract)
            sdev = fold.tile([1, HID], F32, tag="fsd")
            nc.scalar.activation(out=sdev[:], in_=v_row[:], func=AF.Sqrt, bias=eps_t[:])
            rstd = fold.tile([1, HID], F32, tag="frs")
            nc.vector.reciprocal(out=rstd[:], in_=sdev[:])
            g_row = fold.tile([1, HID], F32, tag="fg")
            nc.sync.dma_start(out=g_row[:], in_=win[gname][:])
            a_row = fold.tile([1, HID], F32, tag="fa")
            nc.vector.tensor_tensor(out=a_row[:], in0=g_row[:], in1=rstd[:], op=OP.mult)
            be_row = fold.tile([1, HID], F32, tag="fbe")
            nc.sync.dma_start(out=be_row[:], in_=win[bname][:])
            ma_row = fold.tile([1, HID], F32, tag="fma")
            nc.vector.tensor_tensor(out=ma_row[:], in0=m_row[:], in1=a_row[:], op=OP.mult)
            b_row = fold.tile([1, HID], F32, tag="fb")
            nc.vector.tensor_tensor(out=b_row[:], in0=be_row[:], in1=ma_row[:], op=OP.subtract)

            def to_cols(row, tp2):
                cols = []
                for kci in range(2):
                    pcol = stps.tile([P, 1], F32, space="PSUM", tag="small")
                    nc.tensor.matmul(out=pcol[:], lhsT=row[0:1, kci * P:(kci + 1) * P],
                                     rhs=one_t[:], start=True, stop=True)
                    col = fold.tile([P, 1], F32, tag=f"fc_{tp2}_{kci}")
                    nc.vector.tensor_copy(out=col[:], in_=pcol[:])
                    cols.append(col)
                return cols

            return to_cols(a_row, "a"), to_cols(b_row, "b")

        def stage_weights_folded(li, a_cols, b_cols):
            kc = HID // P
            wraw = fold.tile([P, 2, HID], F32, tag="wraw")
            for wt, name, bias_name in ((wq_f, "Wq", "bq"), (wk_f, "Wk", "k"),
                                        (wv_f, "Wv", "v"), (ws_f, "Ws", "bs")):
                nc.sync.dma_start(
                    out=wraw[:, 0:kc, :],
                    in_=win[f"{name}{li}"][:].rearrange("(c p) g -> p c g", p=P))
                for kci in range(kc):
                    nc.vector.tensor_scalar_mul(out=wt[:, kci, :], in0=wraw[:, kci, :],
                                                scalar1=a_cols[kci][:])
                bps = stps.tile([1, HID], F32, space="PSUM", tag="small")
                for kci in range(kc):
                    nc.tensor.matmul(out=bps[:], lhsT=b_cols[kci][:], rhs=wraw[:, kci, :],
                                     start=(kci == 0), stop=(kci == kc - 1))
                if bias_name in ("bq", "bs"):
                    braw = fold.tile([1, HID], F32, tag="braw")
                    nc.sync.dma_start(out=braw[:], in_=win[f"{bias_name}{li}"][:])
                    tgt = bq_t if bias_name == "bq" else bs_t
                    nc.vector.tensor_tensor(out=tgt[:], in0=braw[:], in1=bps[:], op=OP.add)
                else:
                    wet = wek_t if bias_name == "k" else wev_t
                    src_name = f"Wek{li}" if bias_name == "k" else f"Wev{li}"
                    nc.sync.dma_start(out=wet[:], in_=win[src_name][:])
                    braw2 = fold.tile([1, HID], F32, tag="braw2")
                    nc.sync.dma_start(out=braw2[:],
                                      in_=win[src_name][EDGE_DIM:EDGE_DIM + 1, :])
                    brow = fold.tile([1, HID], F32, tag="brow2")
                    nc.vector.tensor_tensor(out=brow[:], in0=braw2[:], in1=bps[:],
                                            op=OP.add)
                    nc.sync.dma_start(out=wet[EDGE_DIM:EDGE_DIM + 1, :], in_=brow[:])

        def node_phase_kv(h_src, fi):
            kc = fi // P

            def tile_body(load_fn, store_fn):
                h_t = nodeb.tile([P, fi], F32, tag="hkv")
                load_fn(h_t)
                k_ps = nps.tile([P, 2 * HID], F32, space="PSUM", tag="big", name="k_ps")
                v_ps = nps.tile([P, 2 * HID], F32, space="PSUM", tag="big", name="v_ps")
                for kci in range(kc):
                    tp_ps = nps.tile([P, P], F32, space="PSUM", tag="tp")
                    nc.tensor.transpose(out=tp_ps[:], in_=h_t[:, kci * P:(kci + 1) * P],
                                        identity=ident[:])
                    hT = nodeb.tile([P, P], F32, tag="hT")
                    nc.vector.tensor_copy(out=hT[:], in_=tp_ps[:])
                    nc.tensor.matmul(out=k_ps[:, 0:HID], lhsT=hT[:], rhs=wk_f[:, kci, :],
                                     start=(kci == 0), stop=(kci == kc - 1))
                    nc.tensor.matmul(out=v_ps[:, 0:HID], lhsT=hT[:],
                                     rhs=wv_f[:, kci, :],
                                     start=(kci == 0), stop=(kci == kc - 1))
                kv_sb = nodeb.tile([P, 2 * HID], F32, tag="kvsb")
                nc.vector.tensor_copy(out=kv_sb[:, 0:HID], in_=k_ps[:, 0:HID])
                nc.vector.tensor_copy(out=kv_sb[:, HID:2 * HID], in_=v_ps[:, 0:HID])
                store_fn(kv_sb)

            def body(i):
                tile_body(
                    lambda h_t: nc.sync.dma_start(out=h_t[:],
                                                  in_=h_src[bass.ds(i * P, P), :]),
                    lambda kv_sb: nc.sync.dma_start(out=kv_table[bass.ds(i * P, P), :],
                                                    in_=kv_sb[:]))

            tc.For_i_unrolled(0, NT_full, 1, body, max_unroll=8)
            if rem_rows:
                def load_rem(h_t):
                    nc.any.memset(h_t[:], 0.0)
                    nc.sync.dma_start(out=h_t[0:rem_rows, :],
                                      in_=h_src[NT_full * P:n_nodes, :])

                def store_rem(kv_sb):
                    nc.sync.dma_start(out=kv_table[NT_full * P:n_nodes, :],
                                      in_=kv_sb[0:rem_rows, :])
                tile_body(load_rem, store_rem)

        def node_phase_qs(h_own_src, fi, own_rows):
            kc = fi // P

            def body(i):
                h_t = nodeb.tile([P, fi], F32, tag="hqs")
                nc.sync.dma_start(out=h_t[:], in_=h_own_src[bass.ds(i * P, P), :])
                q_pst = nps.tile([P, 2 * HID], F32, space="PSUM", tag="big", name="q_pst")
                s_pst = nps.tile([P, 2 * HID], F32, space="PSUM", tag="big", name="s_pst")
                q_ps = q_pst[:, 0:HID]
                s_ps = s_pst[:, 0:HID]
                nc.tensor.matmul(out=q_ps, lhsT=ones_row[:], rhs=bq_t[:],
                                 start=True, stop=False)
                nc.tensor.matmul(out=s_ps, lhsT=ones_row[:], rhs=bs_t[:],
                                 start=True, stop=False)
                for kci in range(kc):
                    tp_ps = nps.tile([P, P], F32, space="PSUM", tag="tp")
                    nc.tensor.transpose(out=tp_ps[:], in_=h_t[:, kci * P:(kci + 1) * P],
                                        identity=ident[:])
                    hT = nodeb.tile([P, P], F32, tag="hTqs")
                    nc.vector.tensor_copy(out=hT[:], in_=tp_ps[:])
                    nc.tensor.matmul(out=q_ps, lhsT=hT[:], rhs=wq_f[:, kci, :],
                                     start=False, stop=(kci == kc - 1))
                    nc.tensor.matmul(out=s_ps, lhsT=hT[:], rhs=ws_f[:, kci, :],
                                     start=False, stop=(kci == kc - 1))
                q_sb = nodeb.tile([P, HID], F32, tag="qsb")
                s_sb = nodeb.tile([P, HID], F32, tag="ssb")
                nc.vector.tensor_copy(out=q_sb[:], in_=q_ps)
                nc.vector.tensor_copy(out=s_sb[:], in_=s_ps)
                nc.sync.dma_start(out=q_sl[bass.ds(i * P, P), :], in_=q_sb[:])
                nc.sync.dma_start(out=s_sl[bass.ds(i * P, P), :], in_=s_sb[:])

            # h_own_src has SLP (padded) rows for x_own; ag_in has SL rows:
            # roll full tiles, handle the SL%P tail statically
            nfull = own_rows // P
            tc.For_i_unrolled(0, nfull, 1, body, max_unroll=8)
            if own_rows < SLP:
                i0 = nfull
                h_t = nodeb.tile([P, fi], F32, tag="hqs")
                nc.any.memset(h_t[:], 0.0)
                nc.sync.dma_start(out=h_t[0:own_rows - nfull * P, :],
                                  in_=h_own_src[nfull * P:own_rows, :])
                q_pst = nps.tile([P, 2 * HID], F32, space="PSUM", tag="big", name="q_pst")
                s_pst = nps.tile([P, 2 * HID], F32, space="PSUM", tag="big", name="s_pst")
                q_ps = q_pst[:, 0:HID]
                s_ps = s_pst[:, 0:HID]
                nc.tensor.matmul(out=q_ps, lhsT=ones_row[:], rhs=bq_t[:],
                                 start=True, stop=False)
                nc.tensor.matmul(out=s_ps, lhsT=ones_row[:], rhs=bs_t[:],
                                 start=True, stop=False)
                for kci in range(kc):
                    tp_ps = nps.tile([P, P], F32, space="PSUM", tag="tp")
                    nc.tensor.transpose(out=tp_ps[:], in_=h_t[:, kci * P:(kci + 1) * P],
                                        identity=ident[:])
                    hT = nodeb.tile([P, P], F32, tag="hTqs")
                    nc.vector.tensor_copy(out=hT[:], in_=tp_ps[:])
                    nc.tensor.matmul(out=q_ps, lhsT=hT[:], rhs=wq_f[:, kci, :],
                                     start=False, stop=(kci == kc - 1))
                    nc.tensor.matmul(out=s_ps, lhsT=hT[:], rhs=ws_f[:, kci, :],
                                     start=False, stop=(kci == kc - 1))
                q_sb = nodeb.tile([P, HID], F32, tag="qsb")
                s_sb = nodeb.tile([P, HID], F32, tag="ssb")
                nc.vector.tensor_copy(out=q_sb[:], in_=q_ps)
                nc.vector.tensor_copy(out=s_sb[:], in_=s_ps)
                nc.sync.dma_start(out=q_sl[i0 * P:(i0 + 1) * P, :], in_=q_sb[:])
                nc.sync.dma_start(out=s_sl[i0 * P:(i0 + 1) * P, :], in_=s_sb[:])

        def _tile_epilogue(li, t, acc_ps, stats_all):
            nrows = last_rows if t == TPC - 1 else P
            den_t = edgeb.tile([P, HEADS], F32, tag="den")
            nc.vector.tensor_scalar_add(out=den_t[:], in0=acc_ps[:, HID:HID + HEADS],
                                        scalar1=float(SM_EPS))
            rec_t = edgeb.tile([P, HEADS], F32, tag="rec")
            nc.vector.reciprocal(out=rec_t[:], in_=den_t[:])
            att_t = edgeb.tile([P, HID], F32, tag="att")
            nc.vector.tensor_tensor(
                out=att_t[:].rearrange("p (h d) -> p h d", d=HEAD_DIM),
                in0=acc_ps[:, 0:HID].rearrange("p (h d) -> p h d", d=HEAD_DIM),
                in1=rec_t[:, :, None].to_broadcast([P, HEADS, HEAD_DIM]),
                op=OP.mult)
            s_t = edgeb.tile([P, HID], F32, tag="st_ep")
            nc.sync.dma_start(out=s_t[:], in_=s_sl[t * P:t * P + P, :])
            nc.vector.tensor_tensor(out=att_t[:], in0=att_t[:], in1=s_t[:], op=OP.add)
            lr_t = edgeb.tile([P, HID], F32, tag="lr")
            nc.vector.tensor_scalar_mul(out=lr_t[:], in0=att_t[:], scalar1=float(NEG))
            h_t = edgeb.tile([P, HID], F32, tag="hout")
            nc.vector.tensor_tensor(out=h_t[:], in0=att_t[:], in1=lr_t[:], op=OP.max)
            mask = maskcol[:, 1:2] if t == TPC - 1 else maskcol[:, 0:1]
            sq_t = edgeb.tile([P, 2 * HID], F32, tag="sq")
            nc.vector.tensor_copy(out=sq_t[:, 0:HID], in_=h_t[:])
            nc.vector.tensor_tensor(out=sq_t[:, HID:2 * HID], in0=h_t[:], in1=h_t[:],
                                    op=OP.mult)
            nc.tensor.matmul(out=stats_all[:], lhsT=mask, rhs=sq_t[:],
                             start=(t == 0), stop=(t == TPC - 1))
            nc.sync.dma_start(out=ag_in[li][t * P:t * P + nrows, :], in_=h_t[0:nrows, :])

        def edge_phase(li):
            stats_all = stps.tile([1, 2 * HID], F32, space="PSUM", tag="stat")
            acc = {}
            chunk_in_tile = np.zeros(TPC, np.int64)

            for b in range(NBLK):
                c0 = b * BSZ
                cn = min(BSZ, nch - c0)
                ww = cn * P
                idx_t = edgeb.tile([P, BSZ], I32, tag="idx")
                nc.sync.dma_start(out=idx_t[:, 0:cn], in_=src_pm[:, c0:c0 + cn])
                dix_t = edgeb.tile([P, BSZ], I32, tag="dix")
                nc.sync.dma_start(out=dix_t[:, 0:cn], in_=dstl_pm[:, c0:c0 + cn])
                dsh_t = edgeb.tile([P, BSZ], F32, tag="dsh")
                nc.sync.dma_start(out=dsh_t[:, 0:cn], in_=dsh_pm[:, c0:c0 + cn])
                ea_t = edgeb.tile([EDGE_DIM + 1, BSZ * P], F32, tag="ea")
                nc.sync.dma_start(out=ea_t[:, 0:ww], in_=ea_pm[:, c0 * P:c0 * P + ww])

                kv_t = edgeb.tile([P, BSZ, 2 * HID], F32, tag="kv")
                q_t = edgeb.tile([P, BSZ, HID], F32, tag="q")
                for j in range(cn):
                    nc.gpsimd.indirect_dma_start(
                        out=kv_t[:, j, :], out_offset=None, in_=kv_table[:],
                        in_offset=IndirectOffsetOnAxis(ap=idx_t[:, j:j + 1], axis=0))
                    nc.gpsimd.indirect_dma_start(
                        out=q_t[:, j, :], out_offset=None, in_=q_sl[:],
                        in_offset=IndirectOffsetOnAxis(ap=dix_t[:, j:j + 1], axis=0))
                for j in range(cn):
                    ek_ps = ekvps.tile([P, 2 * HID], F32, space="PSUM", tag="big", name="ek_ps")
                    ev_ps = ekvps.tile([P, 2 * HID], F32, space="PSUM", tag="big", name="ev_ps")
                    nc.tensor.matmul(out=ek_ps[:, 0:HID],
                                     lhsT=ea_t[:, j * P:(j + 1) * P], rhs=wek_t[:],
                                     start=True, stop=True)
                    nc.tensor.matmul(out=ev_ps[:, 0:HID],
                                     lhsT=ea_t[:, j * P:(j + 1) * P], rhs=wev_t[:],
                                     start=True, stop=True)
                    nc.vector.tensor_tensor(out=kv_t[:, j, 0:HID], in0=kv_t[:, j, 0:HID],
                                            in1=ek_ps[:, 0:HID], op=OP.add)
                    nc.vector.tensor_tensor(out=kv_t[:, j, HID:2 * HID],
                                            in0=kv_t[:, j, HID:2 * HID],
                                            in1=ev_ps[:, 0:HID], op=OP.add)
                nc.vector.tensor_tensor(out=q_t[:, 0:cn, :], in0=q_t[:, 0:cn, :],
                                        in1=kv_t[:, 0:cn, 0:HID], op=OP.mult)
                al_t = edgeb.tile([P, BSZ * HEADS], F32, tag="al")
                nc.vector.reduce_sum(
                    out=al_t[:, 0:cn * HEADS],
                    in_=q_t[:, 0:cn, :].rearrange("p c (h d) -> p (c h) d",
                                                  d=HEAD_DIM),
                    axis=mybir.AxisListType.X)
                rhs_t = edgeb.tile([P, BSZ, HID + HEADS], F32, tag="rhs")
                oh_t = edgeb.tile([P, BSZ, P], F32, tag="oh")
                nc.vector.tensor_tensor(
                    out=oh_t[:, 0:cn, :],
                    in0=dsh_t[:, 0:cn, None].to_broadcast([P, cn, P]),
                    in1=iota_t[:, None, :].to_broadcast([P, cn, P]),
                    op=OP.is_equal)
                for j in range(cn):
                    nc.scalar.activation(
                        out=rhs_t[:, j, HID:HID + HEADS],
                        in_=al_t[:, j * HEADS:(j + 1) * HEADS],
                        func=AF.Exp, scale=INV_SQRT_D)
                    nc.vector.tensor_tensor(
                        out=rhs_t[:, j, 0:HID].rearrange("p (h d) -> p h d", d=HEAD_DIM),
                        in0=kv_t[:, j, HID:2 * HID].rearrange("p (h d) -> p h d",
                                                              d=HEAD_DIM),
                        in1=rhs_t[:, j, HID:HID + HEADS][:, :, None].to_broadcast(
                            [P, HEADS, HEAD_DIM]),
                        op=OP.mult)
                    ci = c0 + j
                    t = int(chunk_tile[ci])
                    if t not in acc:
                        acc[t] = accps.tile([P, HID + HEADS], F32, space="PSUM",
                                            tag="acc", name=f"acc_l{li}_t{t}")
                    first = chunk_in_tile[t] == 0
                    last = chunk_in_tile[t] == m_t[t] - 1
                    nc.tensor.matmul(out=acc[t][:], lhsT=oh_t[:, j, :],
                                     rhs=rhs_t[:, j, :],
                                     start=bool(first), stop=bool(last))
                    chunk_in_tile[t] += 1
                    if last:
                        _tile_epilogue(li, t, acc.pop(t), stats_all)

            st_sb = fold.tile([1, 2 * HID], F32, tag="stsb")
            nc.vector.tensor_copy(out=st_sb[:], in_=stats_all[:])
            nc.sync.dma_start(out=ar_in[li][:], in_=st_sb[:])

        # ================= main program =================
        for li in range(n_layers):
            if li == 0:
                stage_weights_raw(0, IN_DIM)
                node_phase_kv(x_full, IN_DIM)
                node_phase_qs(x_own, IN_DIM, SLP)
            else:
                nc.gpsimd.collective_compute(
                    "AllReduce", mybir.AluOpType.add, replica_groups=groups,
                    ins=[ar_in[li - 1][:]], outs=[ar_out[li - 1][:]])
                nc.gpsimd.collective_compute(
                    "AllGather", mybir.AluOpType.bypass, replica_groups=groups,
                    ins=[ag_in[li - 1][:]], outs=[ag_out[li - 1][:]])
                a_cols, b_cols = bn_scalars(ar_out[li - 1], f"g{li - 1}",
                                            f"beta{li - 1}", f"l{li}")
                stage_weights_folded(li, a_cols, b_cols)
                node_phase_kv(ag_out[li - 1], HID)
                node_phase_qs(ag_in[li - 1], HID, SL)
            edge_phase(li)

        li = n_layers - 1
        nc.gpsimd.collective_compute(
            "AllReduce", mybir.AluOpType.add, replica_groups=groups,
            ins=[ar_in[li][:]], outs=[ar_out[li][:]])

        # copy h_last out (debug / with_heads=False correctness output)
        for t in range(TPC):
            nrows = last_rows if t == TPC - 1 else P
            ct = nodeb.tile([P, HID], F32, tag="cpy")
            nc.sync.dma_start(out=ct[0:nrows, :], in_=ag_in[li][t * P:t * P + nrows, :])
            nc.sync.dma_start(out=h_out_ext[t * P:t * P + nrows, :], in_=ct[0:nrows, :])

        if with_heads:
            zpool = ctx.enter_context(tc.tile_pool(name="zpool", bufs=2))
            hbig = ctx.enter_context(tc.tile_pool(name="hbig", bufs=1))
            _heads(nc, tc, mybir, bass, win, ag_in[li], ar_out[li],
                   f"g{li}", f"beta{li}", n_nodes,
                   jl_out, dm_out, ht_out, cst, fold, nodeb, tpps, nps,
                   one_t, ones_row, ident, zpool, hbig)

        ctx.close()

    nc.compile()
    return nc


def _heads(nc, tc, mybir, bass, win, h_src, stats_src, gname, bname, n_total,
           jl_out, dm_out, ht_out, cst, fold, nodeb, tpps, nps, one_t,
           ones_row, ident, zpool, hbig):
    """MLP heads on rows 0:N_JOBS of h_src (valid on core 0 only)."""
    F32 = mybir.dt.float32
    AF = mybir.ActivationFunctionType
    OP = mybir.AluOpType
    NJ = N_JOBS

    # BN fold scalars
    st = fold.tile([1, 2 * HID], F32, tag="hst")
    nc.sync.dma_start(out=st[:], in_=stats_src[:])
    m_row = fold.tile([1, HID], F32, tag="hm")
    nc.scalar.activation(out=m_row[:], in_=st[0:1, 0:HID], func=AF.Copy, scale=1.0 / n_total)
    v_row = fold.tile([1, HID], F32, tag="hv")
    nc.scalar.activation(out=v_row[:], in_=st[0:1, HID:2 * HID], func=AF.Copy, scale=1.0 / n_total)
    msq = fold.tile([1, HID], F32, tag="hmsq")
    nc.vector.tensor_tensor(out=msq[:], in0=m_row[:], in1=m_row[:], op=OP.mult)
    nc.vector.tensor_tensor(out=v_row[:], in0=v_row[:], in1=msq[:], op=OP.subtract)
    eps_h = fold.tile([1, 1], F32, tag="hep")
    nc.any.memset(eps_h[:], BN_EPS)
    sdev = fold.tile([1, HID], F32, tag="hsd")
    nc.scalar.activation(out=sdev[:], in_=v_row[:], func=AF.Sqrt, bias=eps_h[:])
    rstd = fold.tile([1, HID], F32, tag="hrs")
    nc.vector.reciprocal(out=rstd[:], in_=sdev[:])
    g_row = fold.tile([1, HID], F32, tag="hg")
    nc.sync.dma_start(out=g_row[:], in_=win[gname][:])
    a_row = fold.tile([1, HID], F32, tag="ha")
    nc.vector.tensor_tensor(out=a_row[:], in0=g_row[:], in1=rstd[:], op=OP.mult)
    be_row = fold.tile([1, HID], F32, tag="hbe")
    nc.sync.dma_start(out=be_row[:], in_=win[bname][:])
    ma_row = fold.tile([1, HID], F32, tag="hma")
    nc.vector.tensor_tensor(out=ma_row[:], in0=m_row[:], in1=a_row[:], op=OP.mult)
    b_row = fold.tile([1, HID], F32, tag="hb")
    nc.vector.tensor_tensor(out=b_row[:], in0=be_row[:], in1=ma_row[:], op=OP.subtract)

    def to_cols(row, tagp):
        cols = []
        for kci in range(2):
            pcol = tpps.tile([P, 1], F32, space="PSUM", tag="small")
            nc.tensor.matmul(out=pcol[:], lhsT=row[0:1, kci * P:(kci + 1) * P],
                             rhs=one_t[:], start=True, stop=True)
            col = fold.tile([P, 1], F32, tag=f"hc_{tagp}_{kci}")
            nc.vector.tensor_copy(out=col[:], in_=pcol[:])
            cols.append(col)
        return cols

    a_cols = to_cols(a_row, "hac")
    b_cols = to_cols(b_row, "hbc")

    # jobsT [128, 2, NJ]
    jT = cst.tile([P, 2, NJ], F32)
    ntiles = (NJ + P - 1) // P
    for i in range(ntiles):
        rows = min(P, NJ - i * P)
        h_t = nodeb.tile([P, HID], F32, tag="hj")
        if rows < P:
            nc.any.memset(h_t[:], 0.0)
        nc.sync.dma_start(out=h_t[0:rows, :], in_=h_src[i * P:i * P + rows, :])
        for kci in range(2):
            tp_ps = nps.tile([P, P], F32, space="PSUM", tag="tp")
            nc.tensor.transpose(out=tp_ps[:], in_=h_t[:, kci * P:(kci + 1) * P],
                                identity=ident[:])
            nc.vector.tensor_copy(out=jT[:, kci, i * P:i * P + rows],
                                  in_=tp_ps[:, 0:rows])

    def head_layerT(slicer, kc_count, W_name, b_name, fold_w, fout, act, tagp):
        """zo [fout<=128, NJ] = act(W^T z_in + b'). slicer(kci, a, b) -> AP."""
        zo = zpool.tile([P, NJ], F32, tag="zhead", name=f"z{tagp}")
        w_t = fold.tile([P, kc_count, fout], F32, tag="whead", name=f"w{tagp}")
        nc.sync.dma_start(out=w_t[:, 0:kc_count, :],
                          in_=win[W_name][:].rearrange("(c p) g -> p c g", p=P))
        if fold_w:
            for kci in range(kc_count):
                nc.vector.tensor_scalar_mul(out=w_t[:, kci, :], in0=w_t[:, kci, :],
                                            scalar1=a_cols[kci][:])
        b_t = fold.tile([1, fout], F32, tag="fb")
        nc.sync.dma_start(out=b_t[:], in_=win[b_name][:])
        if fold_w:
            wraw = fold.tile([P, kc_count, fout], F32, tag="wrhead", name=f"wr{tagp}")
            nc.sync.dma_start(out=wraw[:, 0:kc_count, :],
                              in_=win[W_name][:].rearrange("(c p) g -> p c g", p=P))
            bps = tpps.tile([1, fout], F32, space="PSUM", tag="small")
            for kci in range(kc_count):
                nc.tensor.matmul(out=bps[:], lhsT=b_cols[kci][:], rhs=wraw[:, kci, :],
                                 start=(kci == 0), stop=(kci == kc_count - 1))
            nc.vector.tensor_tensor(out=b_t[:], in0=b_t[:], in1=bps[:], op=OP.add)
        assert fout <= P
        bcol_ps = tpps.tile([P, 1], F32, space="PSUM", tag="small")
        nc.tensor.matmul(out=bcol_ps[0:fout, :], lhsT=b_t[0:1, 0:fout],
                         rhs=one_t[:], start=True, stop=True)
        bcol = fold.tile([P, 1], F32, tag="hbcol", name=f"bcol{tagp}")
        nc.vector.tensor_copy(out=bcol[0:fout, :], in_=bcol_ps[0:fout, :])
        for nci in range((NJ + 511) // 512):
            w = min(512, NJ - nci * 512)
            zps = nps.tile([P, 512], F32, space="PSUM", tag="big")
            for kci in range(kc_count):
                nc.tensor.matmul(out=zps[0:fout, 0:w], lhsT=w_t[:, kci, :],
                                 rhs=slicer(kci, nci * 512, nci * 512 + w),
                                 start=(kci == 0), stop=(kci == kc_count - 1))
            tmp = nodeb.tile([P, 512], F32, tag="tmh")
            nc.vector.tensor_scalar_add(out=tmp[0:fout, 0:w], in0=zps[0:fout, 0:w],
                                        scalar1=bcol[0:fout, :])
            if act:
                lr = nodeb.tile([P, 512], F32, tag="lrhh")
                nc.vector.tensor_scalar_mul(out=lr[0:fout, 0:w], in0=tmp[0:fout, 0:w],
                                            scalar1=float(NEG))
                nc.vector.tensor_tensor(out=zo[0:fout, nci * 512:nci * 512 + w],
                                        in0=tmp[0:fout, 0:w], in1=lr[0:fout, 0:w],
                                        op=OP.max)
            else:
                nc.vector.tensor_copy(out=zo[0:fout, nci * 512:nci * 512 + w],
                                      in_=tmp[0:fout, 0:w])
        return zo

    jT_slice = lambda kci, a, b: jT[:, kci, a:b]

    # job head
    z1 = head_layerT(jT_slice, 2, "jobW0", "jobb0", True, 2 * HEAD_DIM, True, "j1")
    z1_slice = lambda kci, a, b: z1[:, a:b]
    z2 = head_layerT(z1_slice, 1, "jobW1", "jobb1", False, HEAD_DIM, True, "j2")
    w2_t = hbig.tile([HEAD_DIM, N_JOBS], F32, tag="jw2")
    nc.sync.dma_start(out=w2_t[:], in_=win["jobW2"][:])
    b2_t = hbig.tile([1, N_JOBS], F32, tag="jb2")
    nc.sync.dma_start(out=b2_t[:], in_=win["jobb2"][:])
    for mi in range((NJ + P - 1) // P):
        mrows = min(P, NJ - mi * P)
        for nci in range((NJ + 511) // 512):
            w = min(512, NJ - nci * 512)
            ops = nps.tile([P, 512], F32, space="PSUM", tag="big")
            nc.tensor.matmul(out=ops[0:mrows, 0:w],
                             lhsT=ones_row[0:1, 0:mrows],
                             rhs=b2_t[0:1, nci * 512:nci * 512 + w],
                             start=True, stop=False)
            nc.tensor.matmul(out=ops[0:mrows, 0:w],
                             lhsT=z2[0:HEAD_DIM, mi * P:mi * P + mrows],
                             rhs=w2_t[:, nci * 512:nci * 512 + w],
                             start=False, stop=True)
            ot = nodeb.tile([P, 512], F32, tag="jot")
            nc.vector.tensor_copy(out=ot[0:mrows, 0:w], in_=ops[0:mrows, 0:w])
            nc.sync.dma_start(out=jl_out[mi * P:mi * P + mrows,
                                         nci * 512:nci * 512 + w],
                              in_=ot[0:mrows, 0:w])

    # demand / hot heads
    for hn, out_t in (("demand", dm_out), ("hot", ht_out)):
        za = head_layerT(jT_slice, 2, f"{hn}W0", f"{hn}b0", True, HEAD_DIM, True, hn)
        w1_t = fold.tile([HEAD_DIM, 1], F32, tag=f"{hn}w1")
        nc.sync.dma_start(out=w1_t[:], in_=win[f"{hn}W1"][:])
        b1_t = fold.tile([1, 1], F32, tag=f"{hn}b1")
        nc.sync.dma_start(out=b1_t[:], in_=win[f"{hn}b1"][:])
        bcol_ps = tpps.tile([P, 1], F32, space="PSUM", tag="small")
        nc.tensor.matmul(out=bcol_ps[:], lhsT=ones_row[:], rhs=b1_t[:],
                         start=True, stop=True)
        bcol = fold.tile([P, 1], F32, tag=f"{hn}bcol")
        nc.vector.tensor_copy(out=bcol[:], in_=bcol_ps[:])
        for mi in range((NJ + P - 1) // P):
            mrows = min(P, NJ - mi * P)
            ops = tpps.tile([P, 1], F32, space="PSUM", tag="small")
            nc.tensor.matmul(out=ops[0:mrows, :],
                             lhsT=za[0:HEAD_DIM, mi * P:mi * P + mrows],
                             rhs=w1_t[:], start=True, stop=True)
            ot = nodeb.tile([P, 1], F32, tag=f"{hn}ot")
            nc.scalar.activation(out=ot[0:mrows, :], in_=ops[0:mrows, :],
                                 func=AF.Sigmoid, bias=bcol[0:mrows, :])
            nc.sync.dma_start(out=out_t[mi * P:mi * P + mrows, :], in_=ot[0:mrows, :])


# ---------------- host-side input assembly ----------------
def make_input_maps(x, edge_index, edge_attr, params, meta, per_core,
                    n_layers=3, with_heads=True):
    pr = _np_params(params)
    x = np.ascontiguousarray(np.asarray(x), dtype=np.float32)
    SL, TPC = meta["SL"], meta["TPC"]
    SLP = TPC * P
    n_cores = meta["n_cores"]

    iota = np.ascontiguousarray(np.tile(np.arange(P, dtype=np.float32), (P, 1)))
    ident = np.eye(P, dtype=np.float32)
    maskcol = np.zeros((P, 2), np.float32)
    maskcol[:, 0] = 1.0
    last_rows = SL - (TPC - 1) * P
    maskcol[:last_rows, 1] = 1.0

    common = {"x_full": x, "iota_in": iota, "ident_in": ident,
              "maskcol_in": maskcol}
    convs = ["conv1", "conv2", "conv3"][:n_layers]
    for li, cn in enumerate(convs):
        cp = pr[cn]
        for wn in ("Wq", "Wk", "Wv", "Ws"):
            common[f"{wn}{li}"] = cp[wn]
        common[f"bq{li}"] = cp["bq"].reshape(1, -1)
        common[f"bs{li}"] = cp["bs"].reshape(1, -1)
        common[f"Wek{li}"] = np.ascontiguousarray(
            np.concatenate([cp["We"], cp["bk"].reshape(1, -1)], 0))
        common[f"Wev{li}"] = np.ascontiguousarray(
            np.concatenate([cp["We"], cp["bv"].reshape(1, -1)], 0))
        bn = pr[f"bn{li + 1}"]
        common[f"g{li}"] = bn["g"].reshape(1, -1)
        common[f"beta{li}"] = bn["b"].reshape(1, -1)
    if with_heads:
        for hn in ("job", "demand", "hot"):
            for i, lp in enumerate(pr[hn]):
                common[f"{hn}W{i}"] = lp["W"]
                common[f"{hn}b{i}"] = lp["b"].reshape(1, -1)

    in_maps = []
    for c in range(n_cores):
        m = dict(common)
        xo = np.zeros((SLP, IN_DIM), np.float32)
        xo[:SL] = x[c * SL:(c + 1) * SL]
        m["x_own"] = xo
        m.update(per_core[c])
        in_maps.append(m)
    return in_maps


# ---------------- public entry point ----------------
def kernel(x, edge_index, edge_attr, params):
    from concourse.bass_utils import run_bass_kernel_spmd

    meta, per_core = _prep_edges(edge_index, edge_attr, N_NODES, NC)
    nc = build_kernel(meta, n_layers=3, with_heads=True)
    in_maps = make_input_maps(x, edge_index, edge_attr, params, meta, per_core)
    res = run_bass_kernel_spmd(nc, in_maps, core_ids=list(range(NC)))
    r0 = res.results[0]
    job_logits = np.asarray(r0["job_logits"]).reshape(N_JOBS, N_JOBS)
    demand = np.asarray(r0["demand"]).reshape(N_JOBS, 1)
    hot = np.asarray(r0["hot"]).reshape(N_JOBS, 1)
    return job_logits, demand, hot
